# revision 1
# baseline (speedup 1.0000x reference)
"""Trainium2 Bass kernel for BaselineMultiStepRNN.

Math (per original reference, 1-based step index t = 1..T):
    h_t   = tanh(Wx x_t + Wc cap_{t-1} + Whh h_{t-1} + b_ih + b_hh)
    drop_t = fc_w h_t + fc_b
    cap_t = cap_{t-1} - drop_t ;  out[:, t-1] = cap_t

Folded form used on device (state v_t = cap_t - fc_b):
    W'  = Whh - outer(Wc, fc_w)     (removes cap's one-step feedback lag)
    pre_t = Wx x_t + b * 1 + Wc v_{t-2} + W' h_{t-1}
    h_t  = tanh(pre_t)
    d_t  = fc_w h_t
    v_t  = (v_{t-1} - fc_b) - d_t          (v_0 = cap_0 - fc_b, v_{-1} = cap_0)
    out[:, t-1] = v_t + fc_b

All matmuls run in native fp32 (4 cycles/row): this recurrence is mildly
chaotic (per-step perturbations amplify ~1e5x over 512 steps), so reduced
precision paths (fp32r ~12-bit mantissa) land ~400x outside the fp32
arithmetic envelope.  The x/ones/v rows ride one K=65 matmul; the bias and
the capacity feedback enter through host-built rows, so each step is just
8 matmuls + 2 tanh halves + 1 vector op.

Layouts (per core, batch slice BC=256): everything runs transposed;
h state [128, 512] with free = kc*256 + batch (kc = hidden-row half);
PSUM pre [128, 256] per output-row half; x host-pretransposed into chunk
tiles [65, 8, BC] = [x rows(63) + ones + v-row, 8 steps, batch].
"""

import os

os.environ.setdefault("MYCRO_LOCAL_CACHE", "1")

from contextlib import ExitStack

import numpy as np

import concourse.tile as tile
from concourse import bacc, mybir
from concourse.alu_op_type import AluOpType
from concourse.bass_utils import run_bass_kernel_spmd

T_FULL = 512
F = 63
H = 256
B_FULL = 2048
NCORES = 8
BC = B_FULL // NCORES  # 256 batch per core
CH = 8                 # time steps per x chunk tile
F32 = mybir.dt.float32

_CACHE: dict = {}


def _build(T: int):
    if T in _CACHE:
        return _CACHE[T]

    NSLOT = T + 2              # slot s holds step s+1's rows; +2 for v tail
    NCHUNK = (NSLOT + CH - 1) // CH
    nc = bacc.Bacc(
        "TRN2", target_bir_lowering=False, debug=False, enable_asserts=False
    )
    xTd = nc.dram_tensor("xT", [NCHUNK, F + 2, CH, BC], F32, kind="ExternalInput").ap()
    wxbvd = nc.dram_tensor("wxbv", [F + 2, H], F32, kind="ExternalInput").ap()
    wpd = nc.dram_tensor("wp", [128, 2, H], F32, kind="ExternalInput").ap()
    fcd = nc.dram_tensor("fct", [128, 2], F32, kind="ExternalInput").ap()
    fcbd = nc.dram_tensor("fcb", [1, 1], F32, kind="ExternalInput").ap()
    vind = nc.dram_tensor("vinit", [2, BC], F32, kind="ExternalInput").ap()
    voutd = nc.dram_tensor("vout", [T, 1, BC], F32, kind="ExternalOutput").ap()

    TANH = mybir.ActivationFunctionType.Tanh
    SUB = AluOpType.subtract
    KV = F + 2  # 65 rows: x(63), ones, v

    with tile.TileContext(nc) as tc, ExitStack() as ctx:
        consts = ctx.enter_context(tc.tile_pool(name="consts", bufs=1))
        wxbv = consts.tile([KV, H], F32)
        wp = consts.tile([128, 2, H], F32)
        fct = consts.tile([128, 2], F32)
        fcb = consts.tile([1, 1], F32)
        vin1 = consts.tile([1, BC], F32)
        nc.sync.dma_start(wxbv[:], wxbvd[:])
        nc.sync.dma_start(wp[:], wpd[:])
        nc.sync.dma_start(fct[:], fcd[:])
        nc.sync.dma_start(fcb[:], fcbd[:])
        nc.sync.dma_start(vin1[:], vind[1:2, :])

        xpool = ctx.enter_context(tc.tile_pool(name="xpool", bufs=4))
        vlpool = ctx.enter_context(tc.tile_pool(name="vlpool", bufs=4))
        hpool = ctx.enter_context(tc.tile_pool(name="hpool", bufs=2))
        ppool = ctx.enter_context(tc.tile_pool(name="ppool", bufs=3, space="PSUM"))
        dpool = ctx.enter_context(tc.tile_pool(name="dpool", bufs=2, space="PSUM"))

        xtiles: dict = {}

        def xchunk(c):
            if c not in xtiles:
                xt = xpool.tile([F + 2, CH, BC], F32, name="xt", tag="xt")
                if c == 0:
                    # includes host v_{-1}, v_0 in row F+1 slots 0,1
                    nc.sync.dma_start(xt[:], xTd[c])
                else:
                    nc.sync.dma_start(xt[0:F + 1], xTd[c, 0:F + 1])
                xtiles[c] = xt
            return xtiles[c]

        def slot_rhs(s):
            return xchunk(s // CH)[:, s % CH, :]

        def vrow(s):  # [1, BC] AP holding v_{s-1} (slot s's v row)
            return xchunk(s // CH)[F + 1:F + 2, s % CH, :]

        h_prev = None
        d = None
        vf: dict = {}  # s -> [1, BC] tile with v_s (partition 0, for DVE/DMA)

        def emit_vsub(s):
            """v_s = (v_{s-1} - fcb) - d_s: once into slot s+1's matmul row,
            once into a partition-0 tile for the chain + output DMA."""
            prev = vin1[:] if s == 1 else vf[s - 1][:]
            nc.vector.scalar_tensor_tensor(
                vrow(s + 1), prev, fcb[0:1, 0:1], d[:], op0=SUB, op1=SUB
            )
            v = vlpool.tile([1, BC], F32, name="v", tag="v")
            nc.vector.scalar_tensor_tensor(
                v[:], prev, fcb[0:1, 0:1], d[:], op0=SUB, op1=SUB
            )
            vf[s] = v
            vf.pop(s - 2, None)
            nc.sync.dma_start(voutd[s - 1], v[:])

        for t in range(1, T + 1):
            hp = [
                ppool.tile([128, BC], F32, name="hp0", tag="hp0"),
                ppool.tile([128, BC], F32, name="hp1", tag="hp1"),
            ]
            first = h_prev is None
            rx = slot_rhs(t - 1)
            # 1) x + bias + capacity matmuls open the PSUM groups (K=65)
            for mt in range(2):
                nc.tensor.matmul(
                    hp[mt][:], wxbv[:, mt * 128:(mt + 1) * 128], rx,
                    start=True, stop=first,
                )
            if not first:
                # 2) recurrent K0 chunk (needs first tanh half of t-1)
                for mt in range(2):
                    nc.tensor.matmul(
                        hp[mt][:],
                        wp[:, 0, mt * 128:(mt + 1) * 128],
                        h_prev[:, 0:BC],
                        start=False, stop=False,
                    )
                # 3) first half of fc for step t-1
                d = dpool.tile([1, BC], F32, name="d", tag="d")
                nc.tensor.matmul(
                    d[:], fct[:, 0:1], h_prev[:, 0:BC], start=True, stop=False
                )
                # 4) recurrent K1 chunk (needs second tanh half of t-1)
                for mt in range(2):
                    nc.tensor.matmul(
                        hp[mt][:],
                        wp[:, 1, mt * 128:(mt + 1) * 128],
                        h_prev[:, BC:2 * BC],
                        start=False, stop=True,
                    )
                # 5) second half of fc for step t-1
                nc.tensor.matmul(
                    d[:], fct[:, 1:2], h_prev[:, BC:2 * BC], start=False, stop=True
                )
            # 6) tanh, split in halves so next step's K0 matmuls start early
            h = hpool.tile([128, 2 * BC], F32, name="h", tag="h")
            nc.scalar.activation(h[:, 0:BC], hp[0][:], TANH)
            nc.scalar.activation(h[:, BC:2 * BC], hp[1][:], TANH)
            # 7) v update for step t-1
            if not first:
                emit_vsub(t - 1)
            h_prev = h

        # tail: fc + v update for step T
        d = dpool.tile([1, BC], F32, name="d", tag="d")
        nc.tensor.matmul(d[:], fct[:, 0:1], h_prev[:, 0:BC], start=True, stop=False)
        nc.tensor.matmul(d[:], fct[:, 1:2], h_prev[:, BC:2 * BC], start=False, stop=True)
        emit_vsub(T)

    nc.compile()
    _CACHE[T] = nc
    return nc


def _prep_maps(x_seq, seed_capacity, W_ih_w, W_ih_b, W_hh_w, W_hh_b, fc_w, fc_b, T):
    x_seq = np.asarray(x_seq, dtype=np.float32)
    seed = np.asarray(seed_capacity, dtype=np.float32).reshape(B_FULL)
    W_ih_w = np.asarray(W_ih_w, dtype=np.float32)
    W_ih_b = np.asarray(W_ih_b, dtype=np.float32)
    W_hh_w = np.asarray(W_hh_w, dtype=np.float32)
    W_hh_b = np.asarray(W_hh_b, dtype=np.float32)
    fc_w = np.asarray(fc_w, dtype=np.float32)
    fc_b = np.asarray(fc_b, dtype=np.float32)

    Wx = W_ih_w[:, :F]            # [H, 63]
    Wc = W_ih_w[:, F]             # [H]
    bvec = W_ih_b + W_hh_b        # [H]
    fcb_val = float(fc_b[0])

    wxbv = np.concatenate(
        [Wx.T, bvec[None, :], Wc[None, :]], axis=0
    ).astype(np.float32)                                         # [65, H]
    Wp = W_hh_w - np.outer(Wc, fc_w[0])
    wp = np.ascontiguousarray(Wp.T.reshape(2, 128, H).transpose(1, 0, 2))
    fct = np.ascontiguousarray(fc_w[0].reshape(2, 128).T)        # [128, 2]
    fcb = np.array([[fcb_val]], dtype=np.float32)

    NSLOT = T + 2
    NCHUNK = (NSLOT + CH - 1) // CH

    in_maps = []
    for c in range(NCORES):
        sl = slice(c * BC, (c + 1) * BC)
        xc = x_seq[sl, :T, :]                                    # [BC, T, F]
        xtr = np.ascontiguousarray(xc.transpose(1, 2, 0))        # [T, F, BC]
        Tp = NCHUNK * CH
        xtr = np.concatenate(
            [xtr, np.zeros((Tp - T, F, BC), np.float32)], axis=0
        )
        xT = np.zeros((NCHUNK, F + 2, CH, BC), np.float32)
        xT[:, :F] = xtr.reshape(NCHUNK, CH, F, BC).transpose(0, 2, 1, 3)
        xT[:, F] = 1.0                                            # ones row
        seedc = seed[sl]                                          # cap_0
        v0 = (seedc - fcb_val).astype(np.float32)
        xT[0, F + 1, 0] = seedc                                   # v_{-1}
        xT[0, F + 1, 1] = v0                                      # v_0
        in_maps.append(
            {
                "xT": np.ascontiguousarray(xT),
                "wxbv": wxbv,
                "wp": wp,
                "fct": fct,
                "fcb": fcb,
                "vinit": np.ascontiguousarray(np.stack([seedc, v0])),
            }
        )
    return in_maps, fcb_val


def _run(trace=False, **inputs):
    T = int(inputs.get("forecast_steps", T_FULL))
    nc = _build(T)
    in_maps, fcb_val = _prep_maps(
        inputs["x_seq"], inputs["seed_capacity"],
        inputs["W_ih_w"], inputs["W_ih_b"],
        inputs["W_hh_w"], inputs["W_hh_b"],
        inputs["fc_w"], inputs["fc_b"], T,
    )
    res = run_bass_kernel_spmd(
        nc, in_maps, core_ids=list(range(NCORES)), trace=trace
    )
    out = np.empty((B_FULL, T), np.float32)
    for c in range(NCORES):
        v = res.results[c]["vout"].reshape(T, BC)
        out[c * BC:(c + 1) * BC] = (v + fcb_val).T
    return out, res


def kernel(**inputs) -> np.ndarray:
    out, _ = _run(trace=False, **inputs)
    return out



# revision 7
# speedup vs baseline: 1.1155x; 1.1155x over previous
"""Trainium2 Bass kernel for BaselineMultiStepRNN — f32r/fp8 split-matmul.

Math (folded form, 1-based step t = 1..T, per original reference):
    pre_t = Wx x_t + b + Wc v_{t-2} + W' h_{t-1},  W' = Whh - outer(Wc, fc)
    h_t = tanh(pre_t); d_t = fc . h_t; v_t = (v_{t-1} - fcb) - d_t
    out[:, t-1] = v_t + fcb;  v_0 = cap0 - fcb, v_{-1} = cap0

Precision scheme (measured HW behavior: f32r matmul rounds BOTH operands
to 11 explicit mantissa bits; engine writes to f32r tiles round to 12;
fp8 DoubleRow matmuls run at 0.5 cycles/row, f32r at 1.0, fp32 at 4.0):

  Every fp32 product A@B is decomposed as Ah@Bh (f32r, both halves exact
  11-bit by construction) plus fp8e5m2 DoubleRow corrections Ah@Bl and
  (Al*2^11)@(Bh*2^-11), which accumulate into the same PSUM group at true
  scale. Host-side operands (x rows, all weights) are split offline; the
  recurrent h is split on-device: hh = ((h+6144)-6144) (exact multiples of
  2^-11, so the PE's 11-bit rounding is the identity), hl8 = e5m2(h-hh),
  hh2m11 = e5m2(hh*2^-11). v rows are grid-split to 0.25 multiples the
  same way.  d = fc.h gets the same 3-product treatment (its error feeds
  the recurrence through cap and dominated the error budget).

Per step: 6 f32r matmuls (256cy) + 6 fp8-DR (128cy) + 2 f32r fc (256cy)
= 3072 PE cycles vs 8192 for the all-fp32 baseline.

Numerics validated in simulation (sim_final.py): 1.9e-3 max rel err vs
fp64 reference (fp32 recurrence itself: 4e-5; gate: 2e-2).

Layouts per core (batch slice BC=256): h state [128, 2, BC] with
[p, i, n] = hidden unit 128*i+p, batch n; x host-pretransposed into chunk
tiles [67, CH, BC] (f32r rows: xh(63), ones, vh, vl, vh2) and fp8 pair
tiles [127, 2, CH, BC] (xl8/xh2m11, xr8, bias-low).
"""

import os

os.environ.setdefault("MYCRO_LOCAL_CACHE", "1")

from contextlib import ExitStack

import numpy as np
import ml_dtypes

import concourse.tile as tile
from concourse import bacc, mybir
from concourse.alu_op_type import AluOpType
from concourse.bass_utils import run_bass_kernel_spmd

T_FULL = 512
F = 63
H = 256
B_FULL = 2048
NCORES = 8
BC = B_FULL // NCORES  # 256 batch per core
CH = 8                 # time steps per x chunk tile
F32 = mybir.dt.float32
F32R = mybir.dt.float32r
FP8 = mybir.dt.float8e5
F8NP = ml_dtypes.float8_e5m2

KC0 = F + 4            # 67 rows: vh(0), x(1-31), vl(32), x(33-63), vh2(64), x(65), ones(66)
ROW_VH, ROW_VL, ROW_VH2, ROW_ONES = 0, 32, 64, 66
XROWS = list(range(1, 32)) + list(range(33, 64)) + [65]  # x feature j -> XROWS[j]
K8 = 2 * F + 1         # 127 fp8 pair-partitions: xl/xh2m11 (63), xr (63), bias-low

C_H = float(3 * 2.0**11)      # grid constant for h (2^-11 grid)
C_V = float(1.5 * 2.0**21)    # grid constant for v (0.25 grid)

_CACHE: dict = {}


def _build(T: int):
    if T in _CACHE:
        return _CACHE[T]

    NSLOT = T + 2
    NCHUNK = (NSLOT + CH - 1) // CH
    nc = bacc.Bacc(
        "TRN2", target_bir_lowering=False, debug=False, enable_asserts=False
    )
    xmd = nc.dram_tensor("xm", [NCHUNK, KC0, CH, BC], F32R, kind="ExternalInput").ap()
    x8d = nc.dram_tensor("x8", [NCHUNK, K8, 2, CH, BC], FP8, kind="ExternalInput").ap()
    s0d = nc.dram_tensor("s0", [KC0, 2, 128], F32R, kind="ExternalInput").ap()
    s1d = nc.dram_tensor("s1", [128, 2, 128], F32R, kind="ExternalInput").ap()
    s2d = nc.dram_tensor("s2", [128, 2, 128], F32R, kind="ExternalInput").ap()
    w8d0d = nc.dram_tensor("w8d0", [128, 2, 2, 128], FP8, kind="ExternalInput").ap()
    w8d1d = nc.dram_tensor("w8d1", [128, 2, 2, 128], FP8, kind="ExternalInput").ap()
    w8d2d = nc.dram_tensor("w8d2", [K8, 2, 2, 128], FP8, kind="ExternalInput").ap()
    fchtd = nc.dram_tensor("fcht", [128, 2, 16], F32R, kind="ExternalInput").ap()
    f8f2d = nc.dram_tensor("f8f2", [128, 2, 16], FP8, kind="ExternalInput").ap()
    f8f3d = nc.dram_tensor("f8f3", [128, 2, 16], FP8, kind="ExternalInput").ap()
    fcbd = nc.dram_tensor("fcb", [1, 1], F32, kind="ExternalInput").ap()
    vind = nc.dram_tensor("vinit", [1, BC], F32, kind="ExternalInput").ap()
    voutd = nc.dram_tensor("vout", [T, 1, BC], F32, kind="ExternalOutput").ap()

    TANH = mybir.ActivationFunctionType.Tanh
    COPY = mybir.ActivationFunctionType.Copy
    SUB = AluOpType.subtract
    ADD = AluOpType.add
    MULT = AluOpType.mult
    DR = mybir.MatmulPerfMode.DoubleRow

    with tile.TileContext(nc) as tc, ExitStack() as ctx:
        consts = ctx.enter_context(tc.tile_pool(name="consts", bufs=1))
        s0 = consts.tile([KC0, 2, 128], F32R)
        s1 = consts.tile([128, 2, 128], F32R)
        s2 = consts.tile([128, 2, 128], F32R)
        w8d0 = consts.tile([128, 2, 2, 128], FP8)
        w8d1 = consts.tile([128, 2, 2, 128], FP8)
        w8d2 = consts.tile([K8, 2, 2, 128], FP8)
        fcht = consts.tile([128, 2, 16], F32R)
        f8f2 = consts.tile([128, 2, 16], FP8)
        f8f3 = consts.tile([128, 2, 16], FP8)
        fcb = consts.tile([1, 1], F32)
        vin = consts.tile([1, BC], F32)
        for dst, src in [(s0, s0d), (s1, s1d), (s2, s2d), (w8d0, w8d0d),
                         (w8d1, w8d1d), (w8d2, w8d2d), (fcht, fchtd),
                         (f8f2, f8f2d), (f8f3, f8f3d), (fcb, fcbd),
                         (vin, vind)]:
            nc.sync.dma_start(dst[:], src[:])

        xpool = ctx.enter_context(tc.tile_pool(name="xpool", bufs=4))
        x8pool = ctx.enter_context(tc.tile_pool(name="x8pool", bufs=4))
        hpool = ctx.enter_context(tc.tile_pool(name="hpool", bufs=2))
        hhpool = ctx.enter_context(tc.tile_pool(name="hhpool", bufs=2))
        hl8pool = ctx.enter_context(tc.tile_pool(name="hl8pool", bufs=2))
        hh2pool = ctx.enter_context(tc.tile_pool(name="hh2pool", bufs=2))
        vpool = ctx.enter_context(tc.tile_pool(name="vpool", bufs=4))
        ppool = ctx.enter_context(tc.tile_pool(name="ppool", bufs=3, space="PSUM"))
        dpool = ctx.enter_context(tc.tile_pool(name="dpool", bufs=2, space="PSUM"))

        xtiles: dict = {}
        x8tiles: dict = {}

        def xchunk(c):
            if c not in xtiles:
                xt = xpool.tile([KC0, CH, BC], F32R, name="xt", tag="xt")
                nc.sync.dma_start(xt[:], xmd[c])
                xtiles[c] = xt
            return xtiles[c]

        def x8chunk(c):
            if c not in x8tiles:
                xt = x8pool.tile([K8, 2, CH, BC], FP8, name="x8t", tag="x8t")
                nc.sync.dma_start(xt[:], x8d[c])
                x8tiles[c] = xt
            return x8tiles[c]

        def slot_rhs(s):
            return xchunk(s // CH)[:, s % CH, :]

        def slot_rhs8(s):
            return x8chunk(s // CH)[:, :, s % CH, :]

        h_prev = hh_prev = hl8_prev = hh2_prev = None
        d = None
        v_cur = None  # tile holding latest v (v_s)

        def emit_v(s):
            """v_s = (v_{s-1} - fcb) - d_s; write output + v rows for slot s+1."""
            nonlocal v_cur
            prev = vin[:] if s == 1 else v_cur[:]
            v = vpool.tile([1, BC], F32, name="v", tag="v")
            nc.vector.scalar_tensor_tensor(
                v[:], prev, fcb[0:1, 0:1], d[0:1, :], op0=SUB, op1=SUB
            )
            nc.sync.dma_start(voutd[s - 1], v[:])
            sl = s + 1
            xt = xchunk(sl // CH)
            vhr = xt[ROW_VH:ROW_VH + 1, sl % CH, :]
            nc.vector.tensor_scalar(vhr, v[:], C_V, C_V, op0=ADD, op1=SUB)
            nc.scalar.copy(
                xt[ROW_VH2:ROW_VH2 + 1, sl % CH, :], vhr.bitcast(F32)
            )
            nc.gpsimd.tensor_tensor(
                xt[ROW_VL:ROW_VL + 1, sl % CH, :], v[:], vhr.bitcast(F32), op=SUB
            )
            v_cur = v

        def emit_fc(hh_s, hl8_s, hh2_s):
            nonlocal d
            d = dpool.tile([16, BC], F32, name="d", tag="d")
            nc.tensor.matmul(d[:], fcht[:, 0, :], hh_s[:, 0, :], start=True, stop=False)
            nc.tensor.matmul(d[:], fcht[:, 1, :], hh_s[:, 1, :], start=False, stop=False)
            nc.tensor.matmul(d[:], f8f2[:], hl8_s[:], start=False, stop=False,
                             perf_mode=DR)
            nc.tensor.matmul(d[:], f8f3[:], hh2_s[:], start=False, stop=True,
                             perf_mode=DR)

        for t in range(1, T + 1):
            first = h_prev is None
            hp = [
                ppool.tile([128, BC], F32, name="hp0", tag="hp0"),
                ppool.tile([128, BC], F32, name="hp1", tag="hp1"),
            ]
            rx = slot_rhs(t - 1)
            rx8 = slot_rhs8(t - 1)
            for mt in range(2):
                nc.tensor.matmul(hp[mt][:], s0[:, mt, :], rx, start=True, stop=False)
            for mt in range(2):
                nc.tensor.matmul(hp[mt][:], w8d2[:, :, mt, :], rx8,
                                 start=False, stop=first, perf_mode=DR)
            if not first:
                for mt in range(2):
                    nc.tensor.matmul(hp[mt][:], s1[:, mt, :], hh_prev[:, 0, :],
                                     start=False, stop=False)
                for mt in range(2):
                    nc.tensor.matmul(hp[mt][:], s2[:, mt, :], hh_prev[:, 1, :],
                                     start=False, stop=False)
                emit_fc(hh_prev, hl8_prev, hh2_prev)
                for mt in range(2):
                    nc.tensor.matmul(hp[mt][:], w8d0[:, :, mt, :], hl8_prev[:],
                                     start=False, stop=False, perf_mode=DR)
                for mt in range(2):
                    nc.tensor.matmul(hp[mt][:], w8d1[:, :, mt, :], hh2_prev[:],
                                     start=False, stop=True, perf_mode=DR)
                emit_v(t - 1)

            h = hpool.tile([128, 2, BC], F32, name="h", tag="h")
            nc.scalar.activation(h[:, 0, :], hp[0][:], TANH)
            nc.scalar.activation(h[:, 1, :], hp[1][:], TANH)
            hh = hhpool.tile([128, 2, BC], F32R, name="hh", tag="hh")
            for i in range(2):
                nc.vector.tensor_scalar(
                    hh[:, i, :], h[:, i, :], C_H, C_H, op0=ADD, op1=SUB
                )
            hl8 = hl8pool.tile([128, 2, BC], FP8, name="hl8", tag="hl8")
            nc.gpsimd.tensor_tensor(
                hl8[:], h[:], hh[:].bitcast(F32), op=SUB
            )
            hh2 = hh2pool.tile([128, 2, BC], FP8, name="hh2", tag="hh2")
            nc.scalar.activation(
                hh2[:], hh[:].bitcast(F32), COPY, scale=float(2.0**-11)
            )
            h_prev, hh_prev, hl8_prev, hh2_prev = h, hh, hl8, hh2

        # tail: fc + v update for step T
        emit_fc(hh_prev, hl8_prev, hh2_prev)
        emit_v(T)

    nc.compile()
    _CACHE[T] = nc
    return nc


def _r11(x):
    """Round fp32 to 11 explicit mantissa bits (ties away from zero)."""
    x = np.asarray(x, np.float32)
    xi = x.view(np.uint32).astype(np.uint64)
    r = ((xi + (1 << 11)) >> 12) << 12
    return r.astype(np.uint32).view(np.float32)


def _e5(x):
    return np.asarray(x, np.float32).astype(F8NP)


def _grid025(v):
    c = np.float32(C_V)
    return np.float32(np.float32(v + c) - c)


def _prep_maps(x_seq, seed_capacity, W_ih_w, W_ih_b, W_hh_b_, W_hh_w, fc_w, fc_b, T):
    x_seq = np.asarray(x_seq, dtype=np.float32)
    seed = np.asarray(seed_capacity, dtype=np.float32).reshape(B_FULL)
    W_ih_w = np.asarray(W_ih_w, dtype=np.float32)
    b = (np.asarray(W_ih_b, dtype=np.float32)
         + np.asarray(W_hh_b_, dtype=np.float32))
    W_hh_w = np.asarray(W_hh_w, dtype=np.float32)
    fc_w = np.asarray(fc_w, dtype=np.float32)
    fc_b = np.asarray(fc_b, dtype=np.float32)

    Wx = W_ih_w[:, :F]            # [H, 63]
    Wc = W_ih_w[:, F]             # [H]
    fc = fc_w[0]                  # [H]
    fcb_val = float(fc_b[0])
    Wp = (W_hh_w - np.outer(Wc, fc)).astype(np.float32)

    # weight splits
    Wph = _r11(Wp)
    Wpl = (Wp - Wph).astype(np.float32)
    Wph8 = _e5(Wph)
    Wpl8s = _e5(Wpl * 2048.0)
    Wxh = _r11(Wx)
    Wxh8 = _e5(Wxh)
    Wxl8s = _e5((Wx - Wxh) * 2048.0)
    Wch = _r11(Wc)
    Wcl = _r11((Wc - Wch).astype(np.float32))
    bh = _r11(b)
    bl8s = _e5((b - bh) * 2048.0)
    fch = _r11(fc)
    fc8 = _e5(fch)
    fcl8s = _e5((fc - fch) * 2048.0)

    # stationaries
    s0 = np.zeros((KC0, 2, 128), np.float32)
    s0[XROWS] = Wxh.T.reshape(F, 2, 128)
    s0[ROW_ONES] = bh.reshape(2, 128)
    s0[ROW_VH] = Wch.reshape(2, 128)
    s0[ROW_VL] = Wch.reshape(2, 128)
    s0[ROW_VH2] = Wcl.reshape(2, 128)
    s1 = np.ascontiguousarray(Wph.T[0:128].reshape(128, 2, 128))
    s2 = np.ascontiguousarray(Wph.T[128:256].reshape(128, 2, 128))

    w8d0 = np.zeros((128, 2, 2, 128), F8NP)
    w8d1 = np.zeros((128, 2, 2, 128), F8NP)
    for i in range(2):
        w8d0[:, i] = Wph8.T[128 * i:128 * (i + 1)].reshape(128, 2, 128)
        w8d1[:, i] = Wpl8s.T[128 * i:128 * (i + 1)].reshape(128, 2, 128)

    w8d2 = np.zeros((K8, 2, 2, 128), F8NP)
    w8d2[:F, 0] = Wxh8.T.reshape(F, 2, 128)
    w8d2[:F, 1] = Wxl8s.T.reshape(F, 2, 128)
    w8d2[F:2 * F, 0] = Wxh8.T.reshape(F, 2, 128)
    w8d2[2 * F, 0] = bl8s.reshape(2, 128)

    fcht = np.zeros((128, 2, 16), np.float32)
    f8f2 = np.zeros((128, 2, 16), F8NP)
    f8f3 = np.zeros((128, 2, 16), F8NP)
    for i in range(2):
        fcht[:, i, 0] = fch[128 * i:128 * (i + 1)]
        f8f2[:, i, 0] = fc8[128 * i:128 * (i + 1)]
        f8f3[:, i, 0] = fcl8s[128 * i:128 * (i + 1)]

    fcb = np.array([[fcb_val]], dtype=np.float32)

    NSLOT = T + 2
    NCHUNK = (NSLOT + CH - 1) // CH
    Tp = NCHUNK * CH

    # x splits (full batch at once)
    xs = x_seq[:, :T, :]                                     # [B, T, F]
    xh = _r11(xs)
    xl8 = _e5(xs - xh)
    xr8 = _e5((xs - xh - xl8.astype(np.float32)).astype(np.float32))
    xh2m11 = _e5(xh * np.float32(2.0**-11))

    in_maps = []
    for c in range(NCORES):
        sl = slice(c * BC, (c + 1) * BC)

        def chunked(a, dtype):
            # [BC, T, F] -> [NCHUNK, F, CH, BC]
            t_ = np.ascontiguousarray(a[sl].transpose(1, 2, 0))  # [T, F, BC]
            t_ = np.concatenate(
                [t_, np.zeros((Tp - T, F, BC), t_.dtype)], axis=0)
            return np.ascontiguousarray(
                t_.reshape(NCHUNK, CH, F, BC).transpose(0, 2, 1, 3)).astype(dtype)

        xm = np.zeros((NCHUNK, KC0, CH, BC), np.float32)
        xm[:, XROWS] = chunked(xh, np.float32)
        xm[:, ROW_ONES] = 1.0
        # host v rows for slots 0 (v_{-1}=cap0) and 1 (v_0)
        seedc = seed[sl]
        v0 = (seedc - np.float32(fcb_val)).astype(np.float32)
        for slot, vv in ((0, seedc), (1, v0)):
            vh = _grid025(vv)
            vl = (vv - vh).astype(np.float32)
            xm[0, ROW_VH, slot] = vh
            xm[0, ROW_VL, slot] = vl
            xm[0, ROW_VH2, slot] = vh

        x8 = np.zeros((NCHUNK, K8, 2, CH, BC), F8NP)
        x8[:, :F, 0] = chunked(xl8.astype(np.float32), F8NP)
        x8[:, :F, 1] = chunked(xh2m11.astype(np.float32), F8NP)
        x8[:, F:2 * F, 0] = chunked(xr8.astype(np.float32), F8NP)
        x8[:, 2 * F, 0] = np.float32(2.0**-11)

        in_maps.append(
            {
                "xm": np.ascontiguousarray(xm),
                "x8": np.ascontiguousarray(x8),
                "s0": s0, "s1": s1, "s2": s2,
                "w8d0": w8d0, "w8d1": w8d1, "w8d2": w8d2,
                "fcht": fcht, "f8f2": f8f2, "f8f3": f8f3,
                "fcb": fcb,
                "vinit": v0.reshape(1, BC),
            }
        )
    return in_maps, fcb_val


def _run(trace=False, **inputs):
    T = int(inputs.get("forecast_steps", T_FULL))
    nc = _build(T)
    in_maps, fcb_val = _prep_maps(
        inputs["x_seq"], inputs["seed_capacity"],
        inputs["W_ih_w"], inputs["W_ih_b"],
        inputs["W_hh_b"], inputs["W_hh_w"],
        inputs["fc_w"], inputs["fc_b"], T,
    )
    res = run_bass_kernel_spmd(
        nc, in_maps, core_ids=list(range(NCORES)), trace=trace
    )
    out = np.empty((B_FULL, T), np.float32)
    for c in range(NCORES):
        v = res.results[c]["vout"].reshape(T, BC)
        out[c * BC:(c + 1) * BC] = (v + np.float32(fcb_val)).T
    return out, res


def kernel(**inputs) -> np.ndarray:
    out, _ = _run(trace=False, **inputs)
    return out
